# revision 4
# baseline (speedup 1.0000x reference)
"""Horizontal correlation cost volume on 8 Trainium2 NeuronCores.

out[b, ctr, h, w] = sum_c a[b, c, h, w] * b_[b, c, h, w - (D - ctr)],  D = 40.

Sharding: data-parallel over batch B=8, one batch element per core.

Per-core device algorithm (a_i, b_i: [C=128, H=192, W=256] fp32):
  For each h row and each 128-wide w tile, 4 column-tiled fp32 matmuls
  (tile_position col groups g) compute a compact displacement band
    psum[32g + m0, j] = sum_c a[c, w0 + 32g + m0] * b[c, w0 + 32g + j - 40]
  for j in [0,72); the 41 displacement values for output column w = w0+32g+m0
  sit at j = m0..m0+40 of partition 32g+m0.  Out-of-image b columns (only the
  first w-tile's groups g=0,1) are skipped by clipping the moving operand; the
  affected psum region is garbage and the host zeroes the corresponding
  (w + ctr < 40) output triangle, which is exactly zero by definition.

  The per-partition diagonal band cannot be extracted on-device with access
  patterns (per-partition byte offsets are unsupported by the DMA AP
  lowering, and engine APs are partition-uniform), so the band tiles are
  written rectangularly to DRAM outputs and the host performs the final
  diagonal re-indexing (a pure layout gather of device-computed values).
"""
import sys

if "/opt/trn_rl_repo" not in sys.path:
    sys.path.insert(0, "/opt/trn_rl_repo")

import numpy as np

C, H, W, D = 128, 192, 256, 40
DCT = D + 1          # 41 displacements
T = 128              # w-tile width (psum partitions)
R = 16               # h rows per strip
G = 4                # col-tile groups per w-tile
GW = T // G          # 32 output columns per group
NJ = GW + D          # 72 band columns per group
NSTRIP = H // R      # 12
WT = W // T          # 2
NBUF = 4             # strip pipeline depth

_CACHE = {}


def IN_DT(mybir):
    """DRAM input dtype (shared with timing_v6)."""
    return mybir.dt.float32


def make_stage_tensors(nc, mybir, kind):
    """Stage-output DRAM tensors (shared with timing_v6)."""
    f32 = mybir.dt.float32
    return [
        [nc.dram_tensor(f"st_{s}_{w}", [C, R, NJ], f32, kind=kind)
         for w in range(WT)]
        for s in range(NSTRIP)
    ]


def _emit(nc, tc, a_d, b_d, stages):
    """Emit the per-core device program body (shared with the timing build)."""
    import concourse.mybir as mybir

    f32 = mybir.dt.float32
    with (
        tc.tile_pool(name="persist", bufs=1) as pp,
        tc.tile_pool(name="ps", bufs=8, space="PSUM") as psp,
    ):
        A_sb = [pp.tile([C, R, W], f32, tag=f"a{k}", name=f"a{k}")
                for k in range(NBUF)]
        B_sb = [pp.tile([C, R, W], f32, tag=f"b{k}", name=f"b{k}")
                for k in range(NBUF)]
        S_sb = [pp.tile([C, WT * R, NJ], f32, tag=f"s{k}", name=f"s{k}")
                for k in range(NBUF)]

        for s in range(NSTRIP):
            k = s % NBUF
            h0 = s * R
            hh = R // 2
            nc.sync.dma_start(A_sb[k][:, 0:hh, :], a_d.ap()[:, h0:h0 + hh, :])
            nc.scalar.dma_start(B_sb[k][:, 0:hh, :], b_d.ap()[:, h0:h0 + hh, :])
            nc.sync.dma_start(A_sb[k][:, hh:R, :], a_d.ap()[:, h0 + hh:h0 + R, :])
            nc.scalar.dma_start(B_sb[k][:, hh:R, :], b_d.ap()[:, h0 + hh:h0 + R, :])
            for wt in range(WT):
                for h in range(R):
                    psum = psp.tile([C, NJ], f32)
                    for g in range(G):
                        bcol0 = wt * T + GW * g - D   # first b col of group
                        clip = max(0, -bcol0)
                        nc.tensor.matmul(
                            psum[GW * g:GW * (g + 1), clip:NJ],
                            A_sb[k][:, h, wt * T + GW * g: wt * T + GW * (g + 1)],
                            B_sb[k][:, h, bcol0 + clip: bcol0 + NJ],
                            start=True, stop=True,
                            tile_position=(0, GW * g),
                        )
                    nc.vector.tensor_copy(S_sb[k][:, wt * R + h, :], psum[:])
                st_eng = nc.sync if wt == 0 else nc.scalar
                st_eng.dma_start(
                    stages[s][wt].ap(), S_sb[k][:, wt * R:(wt + 1) * R, :]
                )


def _build():
    import concourse.bacc as bacc
    import concourse.mybir as mybir
    import concourse.tile as tile

    nc = bacc.Bacc("TRN2", target_bir_lowering=False, debug=False, num_devices=8)
    a_d = nc.dram_tensor("a", [C, H, W], IN_DT(mybir), kind="ExternalInput")
    b_d = nc.dram_tensor("b", [C, H, W], IN_DT(mybir), kind="ExternalInput")
    stages = make_stage_tensors(nc, mybir, kind="ExternalOutput")

    with tile.TileContext(nc) as tc:
        _emit(nc, tc, a_d, b_d, stages)

    nc.compile()
    return nc


def _get_nc():
    if "nc" not in _CACHE:
        _CACHE["nc"] = _build()
    return _CACHE["nc"]


def _assemble(results):
    """Host-side diagonal extraction from the staged band tiles."""
    # st: [8, WT, NSTRIP, C, R, NJ]
    st = np.stack([
        np.stack([
            np.stack([results[i][f"st_{s}_{w}"] for s in range(NSTRIP)])
            for w in range(WT)
        ])
        for i in range(8)
    ])
    st = st.reshape(8, WT, NSTRIP, G, GW, R, NJ)
    m0 = np.arange(GW)
    out = np.empty((8, DCT, NSTRIP, R, WT, G, GW), np.float32)
    for ctr in range(DCT):
        # advanced indexing over (m0-axis4, j-axis6) -> [GW, 8, WT, NSTRIP, G, R]
        dg = st[:, :, :, :, m0, :, m0 + ctr]
        out[:, ctr] = dg.transpose(1, 3, 5, 2, 4, 0)
    out = out.reshape(8, DCT, H, W)
    # zero the w + ctr < 40 triangle (b column out of image)
    wg = np.arange(W)[None, :]
    cg = np.arange(DCT)[:, None]
    mask = (wg + cg) < D                      # [DCT, W]
    return np.where(mask[None, :, None, :], np.float32(0.0), out)


def run(a, b, trace=False):
    """a, b: [8, C, H, W] fp32. Returns (out [8, DCT, H, W], BassKernelResults)."""
    from concourse import bass_utils

    nc = _get_nc()
    a = np.ascontiguousarray(np.asarray(a, dtype=np.float32))
    b = np.ascontiguousarray(np.asarray(b, dtype=np.float32))
    in_maps = [{"a": a[i], "b": b[i]} for i in range(8)]
    res = bass_utils.run_bass_kernel_spmd(
        nc, in_maps, core_ids=list(range(8)), trace=trace
    )
    out = _assemble(res.results)
    return out, res


def kernel(a, b, max_displacement):
    assert int(max_displacement) == D
    out, _ = run(a, b)
    return out



# revision 5
# speedup vs baseline: 1.8586x; 1.8586x over previous
"""Horizontal correlation cost volume on 8 Trainium2 NeuronCores.

out[b, ctr, h, w] = sum_c a[b, c, h, w] * b_[b, c, h, w - (D - ctr)],  D = 40.

Sharding: data-parallel over batch B=8, one batch element per core.

Per-core device algorithm (a_i, b_i: [C=128, H=192, W=256] -> bf16):
  Inputs are converted to bf16 on the host (halves HBM read traffic; the
  fp32 correctness budget here is ~2e-2 rel, bf16 matmul with fp32 psum
  accumulation lands ~4e-3).  For each h row and each 128-wide w tile, 4
  column-tiled bf16 matmuls (tile_position col groups g) compute a compact
  displacement band
    psum[32g + m0, j] = sum_c a[c, w0 + 32g + m0] * b[c, w0 + 32g + j - 40]
  for j in [0,72); the 41 displacement values for output column w = w0+32g+m0
  sit at j = m0..m0+40 of partition 32g+m0.  Out-of-image b columns (only the
  first w-tile's groups g=0,1) are skipped by clipping the moving operand; the
  affected psum region is garbage and the host zeroes the corresponding
  (w + ctr < 40) output triangle, which is exactly zero by definition.

  Four consecutive h rows share one PSUM bank ([128, 4, 72] tile), so PSUM is
  evacuated in [128, 288] chunks, alternating between the Vector and Scalar
  engines (both can read PSUM; the copy also downcasts fp32->bf16).  Band
  tiles are staged to DRAM in bf16 via the gpsimd (SWDGE) DMA ring so store
  traffic never blocks the sync-ring input prefetch; the host performs the
  final diagonal re-indexing (a pure layout gather of device-computed values,
  as per-partition offsets are unsupported by device access patterns).
"""
import sys

if "/opt/trn_rl_repo" not in sys.path:
    sys.path.insert(0, "/opt/trn_rl_repo")

import numpy as np

C, H, W, D = 128, 192, 256, 40
DCT = D + 1          # 41 displacements
T = 128              # w-tile width (psum partitions)
R = 16               # h rows per strip
G = 4                # col-tile groups per w-tile
GW = T // G          # 32 output columns per group
NJ = GW + D          # 72 band columns per group
HQ = 4               # h rows packed per PSUM bank
NSTRIP = H // R      # 12
WT = W // T          # 2
NBUF = 4             # strip pipeline depth

_CACHE = {}


def IN_DT(mybir):
    """DRAM input dtype (shared with timing_v6)."""
    return mybir.dt.bfloat16


def make_stage_tensors(nc, mybir, kind):
    """Stage-output DRAM tensors (shared with timing_v6)."""
    bf16 = mybir.dt.bfloat16
    return [
        nc.dram_tensor(f"st_{s}", [C, WT, R // HQ, HQ, NJ], bf16, kind=kind)
        for s in range(NSTRIP)
    ]


def _emit(nc, tc, a_d, b_d, stages):
    """Emit the per-core device program body (shared with the timing build)."""
    import concourse.mybir as mybir

    f32 = mybir.dt.float32
    bf16 = mybir.dt.bfloat16
    with (
        tc.tile_pool(name="persist", bufs=1) as pp,
        tc.tile_pool(name="ps", bufs=8, space="PSUM") as psp,
    ):
        A_sb = [pp.tile([C, R, W], bf16, tag=f"a{k}", name=f"a{k}")
                for k in range(NBUF)]
        B_sb = [pp.tile([C, R, W], bf16, tag=f"b{k}", name=f"b{k}")
                for k in range(NBUF)]
        S_sb = [pp.tile([C, WT, R // HQ, HQ, NJ], bf16, tag=f"s{k}", name=f"s{k}")
                for k in range(NBUF)]

        for s in range(NSTRIP):
            k = s % NBUF
            h0 = s * R
            nc.sync.dma_start(A_sb[k][:], a_d.ap()[:, h0:h0 + R, :])
            nc.sync.dma_start(B_sb[k][:], b_d.ap()[:, h0:h0 + R, :])
            cnt = 0
            for wt in range(WT):
                for hq in range(R // HQ):
                    psum = psp.tile([C, HQ, NJ], f32)
                    for hl in range(HQ):
                        h = hq * HQ + hl
                        for g in range(G):
                            bcol0 = wt * T + GW * g - D   # first b col of group
                            clip = max(0, -bcol0)
                            nc.tensor.matmul(
                                psum[GW * g:GW * (g + 1), hl, clip:NJ],
                                A_sb[k][:, h, wt * T + GW * g: wt * T + GW * (g + 1)],
                                B_sb[k][:, h, bcol0 + clip: bcol0 + NJ],
                                start=True, stop=True,
                                tile_position=(0, GW * g),
                            )
                    if cnt % 2 == 0:
                        nc.vector.tensor_copy(S_sb[k][:, wt, hq, :, :], psum[:])
                    else:
                        nc.scalar.copy(S_sb[k][:, wt, hq, :, :], psum[:])
                    cnt += 1
            nc.gpsimd.dma_start(stages[s].ap(), S_sb[k][:])


def _build():
    import concourse.bacc as bacc
    import concourse.mybir as mybir
    import concourse.tile as tile

    nc = bacc.Bacc("TRN2", target_bir_lowering=False, debug=False, num_devices=8)
    a_d = nc.dram_tensor("a", [C, H, W], IN_DT(mybir), kind="ExternalInput")
    b_d = nc.dram_tensor("b", [C, H, W], IN_DT(mybir), kind="ExternalInput")
    stages = make_stage_tensors(nc, mybir, kind="ExternalOutput")

    with tile.TileContext(nc) as tc:
        _emit(nc, tc, a_d, b_d, stages)

    nc.compile()
    return nc


def _get_nc():
    if "nc" not in _CACHE:
        _CACHE["nc"] = _build()
    return _CACHE["nc"]


def _assemble(results):
    """Host-side diagonal extraction from the staged band tiles."""
    # st: [8, NSTRIP, C, WT, R//HQ, HQ, NJ] (bf16 -> fp32)
    st = np.stack([
        np.stack([results[i][f"st_{s}"] for s in range(NSTRIP)])
        for i in range(8)
    ]).astype(np.float32)
    # partitions p = 32g + m  ->  [8, s, g, m, wt, hq, hl, j]
    st = st.reshape(8, NSTRIP, G, GW, WT, R // HQ, HQ, NJ)
    m0 = np.arange(GW)
    out = np.empty((8, DCT, NSTRIP, R // HQ, HQ, WT, G, GW), np.float32)
    for ctr in range(DCT):
        # advanced indexing over (m-axis3, j-axis7):
        # dg: [GW, 8, NSTRIP, G, WT, R//HQ, HQ]
        dg = st[:, :, :, m0, :, :, :, m0 + ctr]
        # -> [8, NSTRIP, R//HQ, HQ, WT, G, GW]
        out[:, ctr] = dg.transpose(1, 2, 5, 6, 4, 3, 0)
    # h = 16s + 4hq + hl ; w = 128wt + 32g + m
    out = out.reshape(8, DCT, H, W)
    # zero the w + ctr < 40 triangle (b column out of image)
    wg = np.arange(W)[None, :]
    cg = np.arange(DCT)[:, None]
    mask = (wg + cg) < D                      # [DCT, W]
    return np.where(mask[None, :, None, :], np.float32(0.0), out)


def run(a, b, trace=False):
    """a, b: [8, C, H, W] fp32. Returns (out [8, DCT, H, W], BassKernelResults)."""
    import ml_dtypes
    from concourse import bass_utils

    nc = _get_nc()
    a = np.ascontiguousarray(np.asarray(a)).astype(ml_dtypes.bfloat16)
    b = np.ascontiguousarray(np.asarray(b)).astype(ml_dtypes.bfloat16)
    in_maps = [{"a": a[i], "b": b[i]} for i in range(8)]
    res = bass_utils.run_bass_kernel_spmd(
        nc, in_maps, core_ids=list(range(8)), trace=trace
    )
    out = _assemble(res.results)
    return out, res


def kernel(a, b, max_displacement):
    assert int(max_displacement) == D
    out, _ = run(a, b)
    return out
